# revision 10
# baseline (speedup 1.0000x reference)
"""MoE routing kernel for Trainium2 (8 NeuronCores, batch-parallel).

Problem: nn_MoE_47278999994656.
  x [8, 256, 80, 80] f32 + gate Linear(256->5) + 5 experts
  (residual conv1x1 on each 128-ch half, gated by a sigmoid transform),
  top-1 masked-softmax gate => weights are EXACTLY one-hot, so
  out[b] = expert_{argmax_e logits[b,e]}(x[b]).

Sharding: data-parallel over batch, core i computes batch item i.

Per core:
  - x is transferred as bf16 (host-cast; device compute was already bf16)
    on two HWDGE rings (sync + scalar) => ~9us load instead of ~18us f32.
  - The gate runs as PSUM-accumulated bf16 matmuls chunk-by-chunk under
    the load; junk matmuls keep the PE clock ramped (pstate).
  - The H layer is algebraically folded: H = (Wt1(I+W))@x + (bt1+Wt1 b),
    so D and H matmuls both read x directly (more ILP, less rounding).
  - A (sigmoid arg) stays 128-replicated via the Wt2-replication matmul;
    BOTH halves' sigmoids run as ONE scalar-engine op over a paired
    2-bank PSUM tile.
  - The combine is two fused scalar_tensor_tensor ops on Vector
    ((D_psum + b) * s, reading D straight from PSUM - no staging copy)
    plus one add on GpSimd; relu alternates Scalar/GpSimd.
  - Expert-select is a 5-step multiply-accumulate chain on Vector over a
    fused [128, 5, 516] table (weights + biases in bf16).
  - Output is written bf16 and upcast on host.
"""

import numpy as np

import concourse.bacc as bacc_mod
import concourse.bass as bass
import concourse.mybir as mybir
import concourse.tile as tile
from concourse.bass_utils import run_bass_kernel_spmd

B, C, H, W = 8, 256, 80, 80
HW = H * W          # 6400
HALF = 128
QUARTER = 64
E = 5
NCORES = 8

# expert-layer chunks: 12 x 512 + 1 x 256 (psum bank holds 512 f32)
CHUNKS = [(i * 512, 512) for i in range(12)] + [(6144, 256)]
# input DMA chunks per half (2KB/partition descriptors)
DCH = [(i * 1024, 1024) for i in range(6)] + [(6144, 256)]

# u_all free-dim layout (per expert, partition dim = 128):
#   [0:128)    (I + Wrgb)^T        [c, o]
#   [128:256)  (I + Wtir)^T        [c, o]
#   [256:320)  Vrgb^T = (Wt1(I+Wrgb))^T   [c, m]
#   [320:384)  Vtir^T                      [c, m]
#   [384:512)  Wt2 replicated      rows 0:64 and 64:128 both = rep
# bias table (separate, f32): b_rgb, b_tir, c_stack, bt2
UF = 512
U_RGB = 0
U_TIR = 128
U_VRGB = 256
U_VTIR = 320
U_WT2 = 384

N_JUNK_TAIL = 14    # PE-warmth bridge across gate-finalize + select

F32 = mybir.dt.float32
BF16 = mybir.dt.bfloat16
AX = mybir.AxisListType.X
ALU = mybir.AluOpType
AF = mybir.ActivationFunctionType


def build_nc() -> bass.Bass:
    nc = bacc_mod.Bacc()

    x0_d = nc.dram_tensor("x0", [HALF, HW], BF16, kind="ExternalInput")
    x1_d = nc.dram_tensor("x1", [HALF, HW], BF16, kind="ExternalInput")
    u_d = nc.dram_tensor("u", [HALF, E, UF], BF16, kind="ExternalInput")
    bias_d = nc.dram_tensor("bias", [HALF, E, 4], F32, kind="ExternalInput")
    wg_d = nc.dram_tensor("wg", [HALF, 2, E], BF16, kind="ExternalInput")
    bg_d = nc.dram_tensor("bg", [1, E], F32, kind="ExternalInput")
    out_d = nc.dram_tensor("out", [HALF, HW], BF16, kind="ExternalOutput")

    with tile.TileContext(nc) as tc:
        with (
            tc.tile_pool(name="big", bufs=1) as big,
            tc.tile_pool(name="const", bufs=1) as const,
            tc.tile_pool(name="small", bufs=1) as small,
            tc.tile_pool(name="selp", bufs=2) as selp,
            tc.tile_pool(name="hsbp", bufs=3) as hsbp,
            tc.tile_pool(name="sstp", bufs=3) as sstp,
            tc.tile_pool(name="combp", bufs=3) as combp,
        ):
            # ---- persistent SBUF ----
            xb = big.tile([HALF, 2, HW], BF16)       # 25.6 KB/part
            u_all = const.tile([HALF, E, UF], BF16)  # 5.1 KB/part
            bias_all = const.tile([HALF, E, 4], F32)
            wg = const.tile([HALF, 2, E], BF16)
            bgx = const.tile([1, E], F32)

            t32a = small.tile([32, 32], F32)
            t32b = small.tile([32, 32], F32)
            ones1 = small.tile([1, HALF], F32)
            l51 = small.tile([E, 1], F32)
            lrow = small.tile([1, E], F32)
            lmax = small.tile([1, 1], F32)
            mrow = small.tile([1, E], F32)
            mbc = small.tile([HALF, E], F32)

            # weights on the gpsimd ring (doesn't compete with x rings)
            nc.gpsimd.dma_start(out=u_all[:], in_=u_d[:])
            nc.gpsimd.dma_start(out=bias_all[:], in_=bias_d[:])
            nc.gpsimd.dma_start(out=wg[:], in_=wg_d[:])
            nc.gpsimd.dma_start(out=bgx[:], in_=bg_d[:])

            # dep-free init
            nc.vector.memset(t32a, 0.0)
            nc.vector.memset(ones1, 1.0)

            with (
                tc.tile_pool(name="gps", bufs=1, space="PSUM") as gps,
                tc.tile_pool(name="jps", bufs=1, space="PSUM") as jps,
            ):
                # ---- phase 1: x load (2 rings) + gate + PE warmth ----
                for off, n in DCH:
                    nc.sync.dma_start(
                        out=xb[:, 0, off : off + n], in_=x0_d[:, off : off + n]
                    )
                    nc.scalar.dma_start(
                        out=xb[:, 1, off : off + n], in_=x1_d[:, off : off + n]
                    )

                # gate: yg[5, 512] += WgT_half^T @ xb sub-chunks (PSUM acc)
                gsl = []
                for off, n in DCH:
                    o = off
                    while o < off + n:
                        m = min(512, off + n - o)
                        gsl.append((o, m))
                        o += m
                yg = gps.tile([E, 512], F32, tag="g")
                nmm = 2 * len(gsl)
                k = 0
                for ci, (o, m) in enumerate(gsl):
                    for h in range(2):
                        nc.tensor.matmul(
                            yg[:, 0:m],
                            lhsT=wg[:, h, :],
                            rhs=xb[:, h, o : o + m],
                            start=(k == 0),
                            stop=(k == nmm - 1),
                        )
                        k += 1
                    # one junk matmul per pair keeps the PE clock ramping
                    if ci % 2 == 1:
                        jnk = jps.tile([E, 512], F32, tag="j")
                        nc.tensor.matmul(
                            jnk, lhsT=wg[:, 1, :], rhs=xb[:, 1, o : o + m]
                        )

                # junk bridge across finalize + select
                for j in range(N_JUNK_TAIL):
                    o = (j % 12) * 512
                    jnk = jps.tile([E, 512], F32, tag="j")
                    nc.tensor.matmul(
                        jnk, lhsT=wg[:, 0, :], rhs=xb[:, 0, o : o + 512]
                    )

                # ---- gate finalize -> one-hot mask on 128 partitions ----
                nc.vector.reduce_sum(l51, yg, axis=AX)
                nc.vector.tensor_copy(t32a[0:E, 0:1], l51)
                nc.vector.transpose(t32b, t32a)
                nc.vector.tensor_add(lrow, t32b[0:1, 0:E], bgx[0:1, :])
                nc.vector.reduce_max(lmax, lrow, axis=AX)
                nc.vector.tensor_scalar(
                    out=mrow, in0=lrow, scalar1=lmax, scalar2=None,
                    op0=ALU.is_equal,
                )
                mps = gps.tile([HALF, E], F32, tag="g")
                nc.tensor.matmul(mps, lhsT=ones1, rhs=mrow)
                nc.vector.tensor_copy(mbc, mps)

                # ---- select expert weights (V-engine stt chain) ----
                acc = selp.tile([HALF, UF], BF16, tag="a")
                nc.vector.tensor_scalar_mul(acc, u_all[:, 0, :], mbc[:, 0:1])
                for e in range(1, E):
                    prev = acc
                    acc = selp.tile([HALF, UF], BF16, tag="a")
                    nc.vector.scalar_tensor_tensor(
                        out=acc, in0=u_all[:, e, :], scalar=mbc[:, e : e + 1],
                        in1=prev, op0=ALU.mult, op1=ALU.add,
                    )
                usel = acc
                # ---- select biases (S muls + G adds, parallel to V) ----
                bsel = small.tile([HALF, 4], F32)
                nc.scalar.activation(
                    out=bsel, in_=bias_all[:, 0, :],
                    func=AF.Copy, scale=mbc[:, 0:1],
                )
                for e in range(1, E):
                    btmp = small.tile([HALF, 4], F32, tag=f"btmp{e}")
                    nc.scalar.activation(
                        out=btmp, in_=bias_all[:, e, :],
                        func=AF.Copy, scale=mbc[:, e : e + 1],
                    )
                    nc.gpsimd.tensor_add(bsel, bsel, btmp)

            # ---- phase 2: selected expert, chunk pipeline ----
            with (
                tc.tile_pool(name="dps", bufs=2, space="PSUM") as dps,
                tc.tile_pool(name="hps", bufs=2, space="PSUM") as hps,
                tc.tile_pool(name="aps", bufs=2, space="PSUM") as aps,
            ):
                for ci, (off, n) in enumerate(CHUNKS):
                    # D layer (residual folded: I+W)
                    dr = dps.tile([HALF, 512], F32, tag="d")
                    nc.tensor.matmul(
                        dr[:, 0:n], lhsT=usel[:, U_RGB : U_RGB + HALF],
                        rhs=xb[:, 0, off : off + n],
                    )
                    dt = dps.tile([HALF, 512], F32, tag="d")
                    nc.tensor.matmul(
                        dt[:, 0:n], lhsT=usel[:, U_TIR : U_TIR + HALF],
                        rhs=xb[:, 1, off : off + n],
                    )
                    # H layer direct from x (folded V = Wt1(I+W)), stacked
                    hp = hps.tile([HALF, 512], F32, tag="h")
                    nc.tensor.matmul(
                        hp[0:QUARTER, 0:n],
                        lhsT=usel[:, U_VRGB : U_VRGB + QUARTER],
                        rhs=xb[:, 0, off : off + n],
                    )
                    nc.tensor.matmul(
                        hp[QUARTER:HALF, 0:n],
                        lhsT=usel[:, U_VTIR : U_VTIR + QUARTER],
                        rhs=xb[:, 1, off : off + n],
                        tile_position=(0, QUARTER),
                    )
                    hsb = hsbp.tile([HALF, 512], BF16, tag="hsb")
                    if ci % 2 == 0:
                        nc.scalar.activation(
                            out=hsb[:, 0:n], in_=hp[:, 0:n],
                            func=AF.Relu, bias=bsel[:, 2:3],
                        )
                    else:
                        # gpsimd cannot read PSUM; alternate relu on V
                        nc.vector.tensor_scalar(
                            out=hsb[:, 0:n], in0=hp[:, 0:n],
                            scalar1=bsel[:, 2:3], scalar2=0.0,
                            op0=ALU.add, op1=ALU.max,
                        )
                    # A layer: both halves into one paired 2-bank PSUM tile,
                    # then ONE sigmoid over the [128, 2*n] view
                    a2 = aps.tile([HALF, 2, 512], F32, tag="a")
                    nc.tensor.matmul(
                        a2[:, 0, 0:n],
                        lhsT=usel[0:QUARTER, U_WT2 : U_WT2 + HALF],
                        rhs=hsb[0:QUARTER, 0:n],
                        tile_position=(0, 0),
                    )
                    nc.tensor.matmul(
                        a2[:, 1, 0:n],
                        lhsT=usel[QUARTER:HALF, U_WT2 : U_WT2 + HALF],
                        rhs=hsb[QUARTER:HALF, 0:n],
                        tile_position=(QUARTER, 0),
                    )
                    sst = sstp.tile([HALF, 2, 512], BF16, tag="s")
                    nc.scalar.activation(
                        out=sst[:, :, 0:n], in_=a2[:, :, 0:n],
                        func=AF.Sigmoid, bias=bsel[:, 3:4],
                    )
                    # combine: (D + b) * s, fused, straight from PSUM
                    prt = combp.tile([HALF, 512], BF16, tag="p")
                    nc.vector.scalar_tensor_tensor(
                        out=prt[:, 0:n], in0=dr[:, 0:n], scalar=bsel[:, 0:1],
                        in1=sst[:, 0, 0:n], op0=ALU.add, op1=ALU.mult,
                    )
                    ob = combp.tile([HALF, 512], BF16, tag="o")
                    nc.vector.scalar_tensor_tensor(
                        out=ob[:, 0:n], in0=dt[:, 0:n], scalar=bsel[:, 1:2],
                        in1=sst[:, 1, 0:n], op0=ALU.add, op1=ALU.mult,
                    )
                    oc = combp.tile([HALF, 512], BF16, tag="c")
                    nc.gpsimd.tensor_add(oc[:, 0:n], prt[:, 0:n], ob[:, 0:n])
                    nc.sync.dma_start(
                        out=out_d[:, off : off + n], in_=oc[:, 0:n]
                    )

    nc.compile()
    return nc


def _pack_inputs(x, Wg, bg, Wrgb, brgb, Wtir, btir, Wt1, bt1, Wt2, bt2):
    import ml_dtypes
    eye = np.eye(HALF, dtype=np.float32)
    u = np.zeros((E, HALF, UF), dtype=np.float32)
    for e in range(E):
        Ar = Wrgb[e] + eye                      # [o, c]
        At = Wtir[e] + eye
        u[e, :, U_RGB : U_RGB + HALF] = Ar.T
        u[e, :, U_TIR : U_TIR + HALF] = At.T
        u[e, :, U_VRGB : U_VRGB + QUARTER] = (Wt1[e] @ Ar).T
        u[e, :, U_VTIR : U_VTIR + QUARTER] = (Wt1[e] @ At).T
        rep = np.repeat(Wt2[e, 0][:, None], HALF, axis=1)   # [64, 128]
        u[e, 0:QUARTER, U_WT2 : U_WT2 + HALF] = rep
        u[e, QUARTER:HALF, U_WT2 : U_WT2 + HALF] = rep
    u = np.ascontiguousarray(u.transpose(1, 0, 2)).astype(ml_dtypes.bfloat16)

    bias = np.zeros((E, HALF, 4), dtype=np.float32)
    for e in range(E):
        bias[e, :, 0] = brgb[e]
        bias[e, :, 1] = btir[e]
        bias[e, 0:QUARTER, 2] = bt1[e] + Wt1[e] @ brgb[e]
        bias[e, QUARTER:HALF, 2] = bt1[e] + Wt1[e] @ btir[e]
        bias[e, :, 3] = bt2[e, 0]
    bias = np.ascontiguousarray(bias.transpose(1, 0, 2))

    wgt = Wg.T.astype(np.float32)                   # [256, 5]
    wg_p = np.ascontiguousarray(
        np.stack([wgt[:HALF], wgt[HALF:]], axis=1)
    ).astype(ml_dtypes.bfloat16)                    # [128, 2, 5]
    bgx = np.ascontiguousarray((bg * float(HW))[None, :].astype(np.float32))

    common = {"u": u, "bias": bias, "wg": wg_p, "bg": bgx}
    in_maps = []
    for b in range(B):
        m = dict(common)
        xr = x[b].reshape(C, HW)
        m["x0"] = np.ascontiguousarray(xr[:HALF]).astype(ml_dtypes.bfloat16)
        m["x1"] = np.ascontiguousarray(xr[HALF:]).astype(ml_dtypes.bfloat16)
        in_maps.append(m)
    return in_maps


_NC_CACHE = {}


def _get_nc():
    if "nc" not in _NC_CACHE:
        _NC_CACHE["nc"] = build_nc()
    return _NC_CACHE["nc"]


def kernel(x, Wg, bg, Wrgb, brgb, Wtir, btir, Wt1, bt1, Wt2, bt2, **run_kw):
    nc = _get_nc()
    in_maps = _pack_inputs(
        np.asarray(x), np.asarray(Wg), np.asarray(bg), np.asarray(Wrgb),
        np.asarray(brgb), np.asarray(Wtir), np.asarray(btir),
        np.asarray(Wt1), np.asarray(bt1), np.asarray(Wt2), np.asarray(bt2),
    )
    res = run_bass_kernel_spmd(nc, in_maps, core_ids=list(range(NCORES)), **run_kw)
    out = np.stack(
        [np.asarray(r["out"]).astype(np.float32) for r in res.results], axis=0
    )
    if run_kw:
        kernel.last_results = res
    return out.reshape(B, HALF, H, W)


# revision 47
# speedup vs baseline: 1.2924x; 1.2924x over previous
"""MoE routing kernel for Trainium2 (8 NeuronCores, batch-parallel).

Problem: nn_MoE_47278999994656.
  x [8, 256, 80, 80] f32 + gate Linear(256->5) + 5 experts
  (residual conv1x1 on each 128-ch half, gated by a sigmoid transform),
  top-1 masked-softmax gate => weights are EXACTLY one-hot, so
  out[b] = expert_{argmax_e logits[b,e]}(x[b]).

Sharding: data-parallel over batch, core i computes batch item i.

Per core:
  - x transfers as bf16 (host-cast; device compute was already bf16) on
    two HWDGE rings (sync + scalar): ~9us load instead of ~18us f32.
  - The gate runs as PSUM-accumulated bf16 matmuls chunk-by-chunk under
    the load; junk matmuls bridge PE-idle windows to hold the clock
    p-state up.
  - Expert select = argmax index (iota dot one-hot mask, broadcast via a
    1-col matmul) driving gpsimd indirect_copy gathers of the weight /
    bias tables - no wide select arithmetic on the Vector engine.
  - The H layer is algebraically folded: H = (Wt1(I+W))@x + (bt1+Wt1 b),
    so D and H matmuls both read x directly.
  - A (sigmoid arg) is 128-replicated via the Wt2-replication matmul into
    a paired 2-bank PSUM tile; ONE sigmoid covers both halves.
  - Combine: two fused scalar_tensor_tensor ops on Vector
    ((D_psum + b) * s, straight from PSUM), final add on GpSimd (SBUF
    bf16), output written bf16 and upcast on host.
  - Phase 2 is software-pipelined [relu(k), A(k), sig(k), H(k+1), D(k),
    combine(k)] so each PSUM tile drains within a chunk period:
    dps2+hps2+aps2x2banks = 8 banks exactly.
"""

import numpy as np

import concourse.bacc as bacc_mod
import concourse.bass as bass
import concourse.mybir as mybir
import concourse.tile as tile
from concourse.bass_utils import run_bass_kernel_spmd

B, C, H, W = 8, 256, 80, 80
HW = H * W          # 6400
HALF = 128
QUARTER = 64
E = 5
NCORES = 8

# expert-layer chunks: 12 x 512 + 1 x 256 (psum bank holds 512 f32)
CHUNKS = [(i * 512, 512) for i in range(12)] + [(6144, 256)]
# input DMA chunks per half (3KB/partition descriptors)
DCH = [(i * 1536, 1536) for i in range(4)] + [(6144, 256)]

# Expert tables are gathered with gpsimd ap_gather (16 row-indices of
# 16-wide rows per gather -> 256 contiguous elements). Table layout
# [128, 5*16, 16]: rows 16e..16e+15 hold expert e's 256-wide payload.
#   uD payload:  (I+Wrgb_e)^T [0:128] | (I+Wtir_e)^T [128:256]
#   uHA payload: Vrgb_e^T [0:64] | Vtir_e^T [64:128] | Wt2 rep [128:256]
#   bias payload: b_rgb, b_tir, c_stack, bt2 at cols 0:4 (bf16, upconverted)
NROW = 16 * E

N_JUNK_A = 7        # PE bridge: gate end -> index broadcast matmul
N_JUNK_B = 11       # PE bridge: index matmul -> first D matmul

F32 = mybir.dt.float32
BF16 = mybir.dt.bfloat16
U16 = mybir.dt.uint16
AX = mybir.AxisListType.X
ALU = mybir.AluOpType
AF = mybir.ActivationFunctionType


def build_nc() -> bass.Bass:
    nc = bacc_mod.Bacc()

    x0_d = nc.dram_tensor("x0", [HALF, HW], BF16, kind="ExternalInput")
    x1_d = nc.dram_tensor("x1", [HALF, HW], BF16, kind="ExternalInput")
    ud_d = nc.dram_tensor("ud", [HALF, NROW, 16], BF16, kind="ExternalInput")
    uha_d = nc.dram_tensor("uha", [HALF, NROW, 16], BF16, kind="ExternalInput")
    bias_d = nc.dram_tensor("bias", [HALF, NROW, 16], BF16, kind="ExternalInput")
    poff_d = nc.dram_tensor("poff", [HALF, 1], F32, kind="ExternalInput")
    wg_d = nc.dram_tensor("wg", [HALF, 2, E], BF16, kind="ExternalInput")
    bg_d = nc.dram_tensor("bg", [1, E], F32, kind="ExternalInput")
    iv_d = nc.dram_tensor("iv", [1, E], F32, kind="ExternalInput")
    out_d = nc.dram_tensor("out", [HALF, HW], BF16, kind="ExternalOutput")

    with tile.TileContext(nc) as tc:
        with (
            tc.tile_pool(name="big", bufs=1) as big,
            tc.tile_pool(name="const", bufs=1) as const,
            tc.tile_pool(name="small", bufs=1) as small,
            tc.tile_pool(name="hsbp", bufs=4) as hsbp,
            tc.tile_pool(name="sstp", bufs=4) as sstp,
            tc.tile_pool(name="combp", bufs=4) as combp,
        ):
            # ---- persistent SBUF ----
            xb = big.tile([HALF, 2, HW], BF16)       # 25.6 KB/part
            ud_all = const.tile([HALF, NROW, 16], BF16)
            uha_all = const.tile([HALF, NROW, 16], BF16)
            bias_all = const.tile([HALF, NROW, 16], BF16)
            poff = const.tile([HALF, 1], F32)
            wg = const.tile([HALF, 2, E], BF16)
            bgx = const.tile([1, E], F32)
            iv = const.tile([1, E], F32)

            t32a = small.tile([32, 32], F32)
            t32b = small.tile([32, 32], F32)
            ones1 = small.tile([1, HALF], F32)
            l51 = small.tile([E, 1], F32)
            lrow = small.tile([1, E], F32)
            lmax = small.tile([1, 1], F32)
            mrow = small.tile([1, E], F32)
            mi = small.tile([1, E], F32)
            idxf = small.tile([1, 1], F32)
            idx16 = small.tile([HALF, 1], mybir.dt.int16)
            usel_d = small.tile([HALF, 256], BF16)
            usel_h = small.tile([HALF, 256], BF16)
            bselg = small.tile([HALF, 256], BF16)
            bself = small.tile([HALF, 4], F32)
            sdum = small.tile([1, 1], F32)

            # weights on the gpsimd ring (doesn't compete with x rings)
            nc.gpsimd.dma_start(out=ud_all[:], in_=ud_d[:])
            nc.gpsimd.dma_start(out=uha_all[:], in_=uha_d[:])
            nc.gpsimd.dma_start(out=bias_all[:], in_=bias_d[:])
            nc.gpsimd.dma_start(out=wg[:], in_=wg_d[:])
            nc.gpsimd.dma_start(out=bgx[:], in_=bg_d[:])
            nc.gpsimd.dma_start(out=iv[:], in_=iv_d[:])
            nc.gpsimd.dma_start(out=poff[:], in_=poff_d[:])

            # dep-free init
            nc.vector.memset(t32a, 0.0)
            nc.vector.memset(ones1, 1.0)
            # pin the sigmoid act-func table before phase 2
            nc.scalar.activation(out=sdum, in_=ones1[0:1, 0:1], func=AF.Sigmoid)

            with (
                tc.tile_pool(name="gps", bufs=1, space="PSUM") as gps,
                tc.tile_pool(name="jps", bufs=1, space="PSUM") as jps,
            ):
                # ---- phase 1: x load (2 rings) + gate under the load ----
                for off, n in DCH:
                    nc.sync.dma_start(
                        out=xb[:, 0, off : off + n], in_=x0_d[:, off : off + n]
                    )
                    nc.scalar.dma_start(
                        out=xb[:, 1, off : off + n], in_=x1_d[:, off : off + n]
                    )

                gsl = []
                for off, n in DCH:
                    o = off
                    while o < off + n:
                        m = min(512, off + n - o)
                        gsl.append((o, m))
                        o += m
                yg = gps.tile([E, 512], F32, tag="g")
                nmm = 2 * len(gsl)
                k = 0
                for o, m in gsl:
                    for h in range(2):
                        nc.tensor.matmul(
                            yg[:, 0:m],
                            lhsT=wg[:, h, :],
                            rhs=xb[:, h, o : o + m],
                            start=(k == 0),
                            stop=(k == nmm - 1),
                        )
                        k += 1

                # junk bridge A: gate end -> index broadcast matmul
                for j in range(N_JUNK_A):
                    jnk = jps.tile([E, 512], F32, tag="j")
                    nc.tensor.matmul(
                        jnk, lhsT=wg[:, 0, :],
                        rhs=xb[:, 0, (j % 12) * 512 : (j % 12) * 512 + 512],
                    )

                # ---- gate finalize -> argmax index (V-engine chain) ----
                nc.vector.reduce_sum(l51, yg, axis=AX)
                nc.vector.tensor_copy(t32a[0:E, 0:1], l51)
                nc.vector.transpose(t32b, t32a)
                nc.vector.tensor_add(lrow, t32b[0:1, 0:E], bgx[0:1, :])
                nc.vector.reduce_max(lmax, lrow, axis=AX)
                nc.vector.tensor_scalar(
                    out=mrow, in0=lrow, scalar1=lmax, scalar2=None,
                    op0=ALU.is_equal,
                )
                nc.vector.tensor_mul(mi, mrow, iv)   # iv = 16*e (row index)
                nc.vector.reduce_sum(idxf, mi, axis=AX)
                ibc = gps.tile([HALF, 1], F32, tag="g")
                nc.tensor.matmul(ibc, lhsT=ones1, rhs=idxf)
                # wrapped per-16-partition row indices: partition p holds
                # row 16e + p%16 (poff = p%16)
                nc.vector.tensor_add(idx16, ibc, poff)

                # junk bridge B: index matmul -> first D matmul
                for j in range(N_JUNK_B):
                    jnk = jps.tile([E, 512], F32, tag="j")
                    nc.tensor.matmul(
                        jnk, lhsT=wg[:, 1, :],
                        rhs=xb[:, 1, (j % 12) * 512 : (j % 12) * 512 + 512],
                    )

                # ---- select expert: three gpsimd ap_gathers ----
                nc.gpsimd.ap_gather(
                    usel_d, ud_all, idx16,
                    channels=HALF, num_elems=NROW, d=16, num_idxs=16,
                )
                nc.gpsimd.ap_gather(
                    usel_h, uha_all, idx16,
                    channels=HALF, num_elems=NROW, d=16, num_idxs=16,
                )
                nc.gpsimd.ap_gather(
                    bselg, bias_all, idx16,
                    channels=HALF, num_elems=NROW, d=16, num_idxs=16,
                )
                nc.vector.tensor_copy(bself, bselg[:, 0:4])

            uDr = usel_d[:, 0:HALF]          # (I+Wrgb)^T
            uDt = usel_d[:, HALF : 2 * HALF]  # (I+Wtir)^T
            uVr = usel_h[:, 0:QUARTER]
            uVt = usel_h[:, QUARTER:HALF]
            uW2 = usel_h[:, HALF : 2 * HALF]  # Wt2 replicated
            bsel = bself

            # ---- phase 2: selected expert, software-pipelined chunks ----
            with (
                tc.tile_pool(name="dps", bufs=2, space="PSUM") as dps,
                tc.tile_pool(name="hps", bufs=2, space="PSUM") as hps,
                tc.tile_pool(name="aps", bufs=2, space="PSUM") as aps,
            ):
                nch = len(CHUNKS)
                hp = [None] * nch
                hsb = [None] * nch

                def emit_h(ci):
                    off, n = CHUNKS[ci]
                    hp[ci] = hps.tile([HALF, 512], F32, tag="h", name="hp")
                    nc.tensor.matmul(
                        hp[ci][0:QUARTER, 0:n],
                        lhsT=uVr,
                        rhs=xb[:, 0, off : off + n],
                    )
                    nc.tensor.matmul(
                        hp[ci][QUARTER:HALF, 0:n],
                        lhsT=uVt,
                        rhs=xb[:, 1, off : off + n],
                        tile_position=(0, QUARTER),
                    )

                emit_h(0)
                for ci, (off, n) in enumerate(CHUNKS):
                    # relu(H + c) -> hsb (S)
                    hsb[ci] = hsbp.tile([HALF, 512], BF16, tag="hsb", name="hsb")
                    nc.scalar.activation(
                        out=hsb[ci][:, 0:n], in_=hp[ci][:, 0:n],
                        func=AF.Relu, bias=bsel[:, 2:3],
                    )
                    # A layer: both halves into one paired 2-bank PSUM tile
                    a2 = aps.tile([HALF, 2, 512], F32, tag="a")
                    nc.tensor.matmul(
                        a2[:, 0, 0:n],
                        lhsT=uW2[0:QUARTER, :],
                        rhs=hsb[ci][0:QUARTER, 0:n],
                        tile_position=(0, 0),
                    )
                    nc.tensor.matmul(
                        a2[:, 1, 0:n],
                        lhsT=uW2[QUARTER:HALF, :],
                        rhs=hsb[ci][QUARTER:HALF, 0:n],
                        tile_position=(QUARTER, 0),
                    )
                    # ONE sigmoid over both halves (S)
                    sst = sstp.tile([HALF, 2, 512], BF16, tag="s")
                    nc.scalar.activation(
                        out=sst[:, :, 0:n], in_=a2[:, :, 0:n],
                        func=AF.Sigmoid, bias=bsel[:, 3:4],
                    )
                    # next chunk's H matmuls keep the PE streaming
                    if ci + 1 < nch:
                        emit_h(ci + 1)
                    # D layer late (short PSUM residency)
                    off_, n_ = off, n
                    dr = dps.tile([HALF, 512], F32, tag="d")
                    nc.tensor.matmul(
                        dr[:, 0:n], lhsT=uDr,
                        rhs=xb[:, 0, off : off + n],
                    )
                    dt = dps.tile([HALF, 512], F32, tag="d")
                    nc.tensor.matmul(
                        dt[:, 0:n], lhsT=uDt,
                        rhs=xb[:, 1, off : off + n],
                    )
                    # combine: (D + b) * s fused on V, final add on G
                    prt = combp.tile([HALF, 512], BF16, tag="p")
                    nc.vector.scalar_tensor_tensor(
                        out=prt[:, 0:n], in0=dr[:, 0:n], scalar=bsel[:, 0:1],
                        in1=sst[:, 0, 0:n], op0=ALU.add, op1=ALU.mult,
                    )
                    ob = combp.tile([HALF, 512], BF16, tag="o")
                    nc.vector.scalar_tensor_tensor(
                        out=ob[:, 0:n], in0=dt[:, 0:n], scalar=bsel[:, 1:2],
                        in1=sst[:, 1, 0:n], op0=ALU.add, op1=ALU.mult,
                    )
                    oc = combp.tile([HALF, 512], BF16, tag="c")
                    nc.gpsimd.tensor_add(oc[:, 0:n], prt[:, 0:n], ob[:, 0:n])
                    nc.sync.dma_start(
                        out=out_d[:, off : off + n], in_=oc[:, 0:n]
                    )

    nc.compile()
    return nc


def _pack_inputs(x, Wg, bg, Wrgb, brgb, Wtir, btir, Wt1, bt1, Wt2, bt2):
    import ml_dtypes
    eye = np.eye(HALF, dtype=np.float32)
    # per-expert 256-wide payloads, then sliced into 16 rows of 16
    udp = np.zeros((E, HALF, 256), dtype=np.float32)
    uhp = np.zeros((E, HALF, 256), dtype=np.float32)
    bip = np.zeros((E, HALF, 256), dtype=np.float32)
    for e in range(E):
        Ar = Wrgb[e] + eye                      # [o, c]
        At = Wtir[e] + eye
        udp[e, :, 0:HALF] = Ar.T
        udp[e, :, HALF:] = At.T
        uhp[e, :, 0:QUARTER] = (Wt1[e] @ Ar).T
        uhp[e, :, QUARTER:HALF] = (Wt1[e] @ At).T
        rep = np.repeat(Wt2[e, 0][:, None], HALF, axis=1)   # [64, 128]
        uhp[e, 0:QUARTER, HALF:] = rep
        uhp[e, QUARTER:HALF, HALF:] = rep
        bip[e, :, 0] = brgb[e]
        bip[e, :, 1] = btir[e]
        bip[e, 0:QUARTER, 2] = bt1[e] + Wt1[e] @ brgb[e]
        bip[e, QUARTER:HALF, 2] = bt1[e] + Wt1[e] @ btir[e]
        bip[e, :, 3] = bt2[e, 0]

    def to_table(p):
        # [E, 128, 256] -> [128, 16E, 16]: row 16e+j = payload cols 16j:16j+16
        t = p.reshape(E, HALF, 16, 16).transpose(1, 0, 2, 3).reshape(
            HALF, 16 * E, 16
        )
        return np.ascontiguousarray(t).astype(ml_dtypes.bfloat16)

    ud = to_table(udp)
    uha = to_table(uhp)
    bias = to_table(bip)

    wgt = Wg.T.astype(np.float32)                   # [256, 5]
    wg_p = np.ascontiguousarray(
        np.stack([wgt[:HALF], wgt[HALF:]], axis=1)
    ).astype(ml_dtypes.bfloat16)                    # [128, 2, 5]
    bgx = np.ascontiguousarray((bg * float(HW))[None, :].astype(np.float32))
    # first table-row index of expert e
    ivx = (16.0 * np.arange(E, dtype=np.float32))[None, :]
    # wrapped per-16-partition row indices: partition p gathers row 16e+p%16
    poffx = (np.arange(HALF, dtype=np.float32) % 16)[:, None].copy()

    common = {"ud": ud, "uha": uha, "bias": bias, "wg": wg_p, "bg": bgx,
              "iv": ivx, "poff": poffx}
    in_maps = []
    for b in range(B):
        m = dict(common)
        xr = x[b].reshape(C, HW)
        m["x0"] = np.ascontiguousarray(xr[:HALF]).astype(ml_dtypes.bfloat16)
        m["x1"] = np.ascontiguousarray(xr[HALF:]).astype(ml_dtypes.bfloat16)
        in_maps.append(m)
    return in_maps


_NC_CACHE = {}


def _get_nc():
    if "nc" not in _NC_CACHE:
        _NC_CACHE["nc"] = build_nc()
    return _NC_CACHE["nc"]


def kernel(x, Wg, bg, Wrgb, brgb, Wtir, btir, Wt1, bt1, Wt2, bt2, **run_kw):
    nc = _get_nc()
    in_maps = _pack_inputs(
        np.asarray(x), np.asarray(Wg), np.asarray(bg), np.asarray(Wrgb),
        np.asarray(brgb), np.asarray(Wtir), np.asarray(btir),
        np.asarray(Wt1), np.asarray(bt1), np.asarray(Wt2), np.asarray(bt2),
    )
    res = run_bass_kernel_spmd(nc, in_maps, core_ids=list(range(NCORES)), **run_kw)
    out = np.stack(
        [np.asarray(r["out"]).astype(np.float32) for r in res.results], axis=0
    )
    if run_kw:
        kernel.last_results = res
    return out.reshape(B, HALF, H, W)
